# revision 8
# baseline (speedup 1.0000x reference)
"""Multi-head self-attention (RoPE, causal) on 8 Trainium2 NeuronCores.

Sharding: core c -> (batch = c//2, head-group = c%2 of 8 heads).
Column-parallel wq/wk/wv, row-parallel wo. Each core emits a partial
out^T [f, s]; the host sums the two partials per batch and transposes.

Layouts (all chosen so no on-device transposes are needed):
  XT  [d, s]   (x transposed on host, bf16)
  Q^T/K^T [e, s] per head from matmul(lhsT=wT[d,e], rhs=XT[d,s])
  V   [s, e]   from matmul(lhsT=XT[d,s], rhs=wvT[d,e])
  S^T [j, i] = matmul(lhsT=K^T[e,j], rhs=Q^T[e,i])   (fp32r)
  ctx^T [e, i] = matmul(lhsT=V[j,e], rhs=expS^T[j,i]) (bf16)
  out^T [f, s] = matmul(lhsT=woT[d,f], rhs=ctx^T[d,s]) (fp32r)

RoPE: head dims de-interleaved on host (even dims -> partitions 0..63,
odd -> 64..127 of each head's Q^T/K^T) by permuting wq/wk rows. Then
rot(x) = x*cc + (SP@x)*ss where SP is a signed permutation (matmul) and
cc/ss are host-precomputed cos/sin tables. The 1/sqrt(dk) scale is
applied via the Exp activation's scale field.

Softmax: no max-subtraction (scores are O(1)-scaled; fp32 exp is safe).
Causal masking by block-skipping + one 128x128 triangular mask on
diagonal blocks. Row sums via ones-vector matmul accumulated in PSUM;
normalization multiplies ctx^T by gpsimd-partition-broadcast recip.
"""

import numpy as np
import ml_dtypes

import concourse.bass as bass
import concourse.tile as tile
import concourse.mybir as mybir
from concourse import bacc, bass_utils

F32 = mybir.dt.float32
F32R = mybir.dt.float32r
BF16 = mybir.dt.bfloat16

B = 4
S = 2048
D = 2048
NH = 16
DK = 128
NCORES = 8
HPC = 8            # heads per core
DLOC = HPC * DK    # 1024, local model dims per core
ST = S // 128      # 16 sequence 128-tiles
DT = D // 128      # 16 model-dim 128-tiles
IB = S // 512      # 4 i-blocks of 512
ROPE_THETA = 10000.0
SCALE = float(1.0 / np.sqrt(DK))

_cache = {}


def build_program():
    if "nc" in _cache:
        return _cache["nc"]

    nc = bacc.Bacc("TRN2", target_bir_lowering=False, debug=False,
                   num_devices=NCORES)

    xt = nc.dram_tensor("xt", [D, S], BF16, kind="ExternalInput").ap()
    wq = nc.dram_tensor("wq", [D, DLOC], BF16, kind="ExternalInput").ap()
    wk = nc.dram_tensor("wk", [D, DLOC], BF16, kind="ExternalInput").ap()
    wv = nc.dram_tensor("wv", [D, DLOC], BF16, kind="ExternalInput").ap()
    wo = nc.dram_tensor("wo", [DLOC, D], F32, kind="ExternalInput").ap()
    cct = nc.dram_tensor("cct", [128, S], F32, kind="ExternalInput").ap()
    sst = nc.dram_tensor("sst", [128, S], F32, kind="ExternalInput").ap()
    sperm = nc.dram_tensor("sperm", [128, 128], F32, kind="ExternalInput").ap()
    tri = nc.dram_tensor("tri", [128, 128], BF16, kind="ExternalInput").ap()
    out = nc.dram_tensor("out", [D, S], F32, kind="ExternalOutput").ap()

    with tile.TileContext(nc) as tc:
        with tc.tile_pool(name="dram", bufs=1, space="DRAM") as dram_pool:
            ctx_dram = dram_pool.tile([128, HPC, S], F32R)
            _attention_phase(nc, tc, xt, wq, wk, wv, cct, sst,
                             sperm, tri, ctx_dram)
            _output_phase(nc, tc, wo, ctx_dram, out)

    nc.compile()
    _cache["nc"] = nc
    return nc


def _attention_phase(nc, tc, xt, wq, wk, wv, cct, sst, sperm, tri, ctx_dram):
    with (
        tc.tile_pool(name="xt", bufs=1) as xt_pool,
        tc.tile_pool(name="vsb", bufs=1) as v_pool,
        tc.tile_pool(name="tabs", bufs=1) as tab_pool,
    ):
        # ---- resident loads ----
        xt_sb = xt_pool.tile([128, DT, S], BF16)
        for dt in range(DT):
            nc.sync.dma_start(xt_sb[:, dt, :], xt[dt * 128:(dt + 1) * 128, :])

        cc_sb = tab_pool.tile([128, S], F32, tag="cct")
        ss_sb = tab_pool.tile([128, S], F32, tag="sst")
        nc.sync.dma_start(cc_sb[:], cct)
        nc.sync.dma_start(ss_sb[:], sst)
        sp_stage = tab_pool.tile([128, 128], F32, tag="sperm_stage")
        nc.sync.dma_start(sp_stage[:], sperm)
        sp_sb = tab_pool.tile([128, 128], F32R, tag="sperm")
        nc.scalar.copy(sp_sb[:], sp_stage[:])
        tri_sb = tab_pool.tile([128, 128], BF16, tag="tri")
        nc.sync.dma_start(tri_sb[:], tri)
        ones_sb = tab_pool.tile([128, 1], BF16, tag="ones")
        nc.gpsimd.memset(ones_sb[:], 1.0)

        # ---- V = x @ wv.T for all local heads: V[s, e] bf16 ----
        v_sb = v_pool.tile([128, ST, DLOC], BF16)
        with (
            tc.tile_pool(name="wvs", bufs=1) as wv_pool,
            tc.tile_pool(name="v_ps", bufs=2, space="PSUM") as v_ps_pool,
        ):
            wv_sb = wv_pool.tile([128, DT, DLOC], BF16)
            for dt in range(DT):
                nc.sync.dma_start(wv_sb[:, dt, :],
                                  wv[dt * 128:(dt + 1) * 128, :])
            for st in range(ST):
                for g in range(2):
                    v_ps = v_ps_pool.tile([128, 512], F32, tag="v_ps")
                    for dt in range(DT):
                        nc.tensor.matmul(
                            v_ps[:],
                            xt_sb[:, dt, st * 128:(st + 1) * 128],
                            wv_sb[:, dt, g * 512:(g + 1) * 512],
                            start=(dt == 0), stop=(dt == DT - 1),
                        )
                    nc.scalar.copy(v_sb[:, st, g * 512:(g + 1) * 512], v_ps[:])

        # ---- per-head QK projection + RoPE + attention ----
        with (
            tc.tile_pool(name="wqk", bufs=1) as wqk_pool,
            tc.tile_pool(name="qkraw", bufs=2) as raw_pool,
            tc.tile_pool(name="rqk", bufs=1) as rqk_pool,
            tc.tile_pool(name="exps", bufs=4) as exp_pool,
            tc.tile_pool(name="small", bufs=2) as small_pool,
            tc.tile_pool(name="ctxsb", bufs=3) as ctx_sb_pool,
            tc.tile_pool(name="qk_ps", bufs=1, space="PSUM") as qk_ps_pool,
            tc.tile_pool(name="sw_ps", bufs=1, space="PSUM") as sw_ps_pool,
            tc.tile_pool(name="s_ps", bufs=2, space="PSUM") as s_ps_pool,
            tc.tile_pool(name="ctx_ps", bufs=1, space="PSUM") as ctx_ps_pool,
            tc.tile_pool(name="rs_ps", bufs=1, space="PSUM") as rs_ps_pool,
        ):
            for h in range(HPC):
                wq_sb = wqk_pool.tile([128, DT, DK], BF16, tag="wq")
                wk_sb = wqk_pool.tile([128, DT, DK], BF16, tag="wk")
                for dt in range(DT):
                    nc.sync.dma_start(wq_sb[:, dt, :],
                                      wq[dt * 128:(dt + 1) * 128,
                                         h * DK:(h + 1) * DK])
                    nc.sync.dma_start(wk_sb[:, dt, :],
                                      wk[dt * 128:(dt + 1) * 128,
                                         h * DK:(h + 1) * DK])

                rq = rqk_pool.tile([128, S], F32R, tag="rq")
                rk = rqk_pool.tile([128, S], F32R, tag="rk")
                for (w_sb, r_t) in ((wk_sb, rk), (wq_sb, rq)):
                    for half in range(2):
                        o = half * 1024
                        ps = qk_ps_pool.tile([128, 1024], F32, tag="qk_ps")
                        for dt in range(DT):
                            for cch in range(2):
                                nc.tensor.matmul(
                                    ps[:, cch * 512:(cch + 1) * 512],
                                    w_sb[:, dt, :],
                                    xt_sb[:, dt,
                                          o + cch * 512:o + (cch + 1) * 512],
                                    start=(dt == 0), stop=(dt == DT - 1),
                                )
                        raw = raw_pool.tile([128, 1024], F32R, tag="qkraw")
                        nc.scalar.copy(raw[:], ps[:])
                        swp = sw_ps_pool.tile([128, 1024], F32, tag="sw_ps")
                        for cch in range(2):
                            nc.tensor.matmul(
                                swp[:, cch * 512:(cch + 1) * 512],
                                sp_sb[:],
                                raw[:, cch * 512:(cch + 1) * 512],
                                start=True, stop=True,
                            )
                        t2 = raw_pool.tile([128, 1024], F32, tag="t2")
                        nc.vector.tensor_mul(t2[:], swp[:], ss_sb[:, o:o + 1024])
                        t3 = raw_pool.tile([128, 1024], F32, tag="t3")
                        nc.vector.tensor_mul(t3[:], raw[:].bitcast(F32),
                                             cc_sb[:, o:o + 1024])
                        nc.vector.tensor_add(r_t[:, o:o + 1024], t2[:], t3[:])

                # attention for this head
                for ib in range(IB):
                    i0 = ib * 512
                    ctx_ps = ctx_ps_pool.tile([128, 512], F32, tag="ctx_ps")
                    rs_ps = rs_ps_pool.tile([1, 512], F32, tag="rs_ps")
                    njt = 4 * ib + 4
                    for jt in range(njt):
                        r = jt - 4 * ib  # >=0 on diagonal blocks
                        lo = 128 * r if r >= 0 else 0
                        s_ps = s_ps_pool.tile([128, 512], F32, tag="s_ps")
                        nc.tensor.matmul(
                            s_ps[:],
                            rk[:, jt * 128:(jt + 1) * 128],
                            rq[:, i0:i0 + 512],
                            start=True, stop=True,
                        )
                        es = exp_pool.tile([128, 512], BF16, tag="exps")
                        nc.scalar.activation(es[:, lo:512], s_ps[:, lo:512],
                                             mybir.ActivationFunctionType.Exp,
                                             scale=SCALE)
                        if r >= 0:
                            nc.vector.tensor_mul(es[:, lo:lo + 128],
                                                 es[:, lo:lo + 128], tri_sb[:])
                        first = (jt == 0)
                        last = (jt == njt - 1)
                        nc.tensor.matmul(
                            ctx_ps[:, lo:512],
                            v_sb[:, jt, h * DK:(h + 1) * DK],
                            es[:, lo:512],
                            start=first, stop=last, skip_group_check=True,
                        )
                        nc.tensor.matmul(
                            rs_ps[:, lo:512],
                            ones_sb[:],
                            es[:, lo:512],
                            start=first, stop=last, skip_group_check=True,
                        )
                    recip = small_pool.tile([1, 512], F32, tag="recip")
                    nc.vector.reciprocal(recip[:], rs_ps[:])
                    bc = small_pool.tile([128, 512], F32, tag="bc")
                    nc.gpsimd.partition_broadcast(bc[:], recip[:])
                    ctx_sb = ctx_sb_pool.tile([128, 512], F32R, tag="ctx_sb")
                    nc.vector.tensor_mul(ctx_sb[:], ctx_ps[:], bc[:])
                    nc.sync.dma_start(ctx_dram[:, h, i0:i0 + 512], ctx_sb[:])


def _output_phase(nc, tc, wo, ctx_dram, out):
    NDT = DLOC // 128  # 8
    with (
        tc.tile_pool(name="wos", bufs=1) as wo_pool,
        tc.tile_pool(name="wostage", bufs=2) as wstage_pool,
        tc.tile_pool(name="ctxin", bufs=2) as cin_pool,
        tc.tile_pool(name="outsb", bufs=3) as out_pool,
        tc.tile_pool(name="wo_ps", bufs=4, space="PSUM") as wo_ps_pool,
    ):
        wo_sb = wo_pool.tile([128, NDT, D], F32R)
        for dt in range(NDT):
            wst = wstage_pool.tile([128, D], F32, tag="wst")
            nc.sync.dma_start(wst[:], wo[dt * 128:(dt + 1) * 128, :])
            nc.scalar.copy(wo_sb[:, dt, :], wst[:])
        for sb4 in range(IB):
            s0 = sb4 * 512
            cin = cin_pool.tile([128, NDT, 512], F32R, tag="cin")
            nc.sync.dma_start(cin[:], ctx_dram[:, :, s0:s0 + 512])
            for ft in range(DT):
                ps = wo_ps_pool.tile([128, 512], F32, tag="wo_ps")
                for dt in range(NDT):
                    nc.tensor.matmul(
                        ps[:],
                        wo_sb[:, dt, ft * 128:(ft + 1) * 128],
                        cin[:, dt, :],
                        start=(dt == 0), stop=(dt == NDT - 1),
                    )
                osb = out_pool.tile([128, 512], F32, tag="osb")
                nc.scalar.copy(osb[:], ps[:])
                nc.sync.dma_start(out[ft * 128:(ft + 1) * 128, s0:s0 + 512],
                                  osb[:])


def prepare_in_maps(x, wq, wk, wv, wo):
    """Build the 8 per-core input maps (host-side sharding + tables)."""
    x = np.asarray(x, dtype=np.float32)
    wq = np.asarray(wq, dtype=np.float32)
    wk = np.asarray(wk, dtype=np.float32)
    wv = np.asarray(wv, dtype=np.float32)
    wo = np.asarray(wo, dtype=np.float32)

    # RoPE tables (fp32, matching the reference's fp32 cos/sin)
    f = np.arange(0, DK, 2, dtype=np.float32) / DK          # 2f/d
    inv_freq = (ROPE_THETA ** (-f)).astype(np.float32)      # [64]
    ang = np.arange(S, dtype=np.float32)[:, None] * inv_freq[None, :]
    cos_t = np.cos(ang).T.astype(np.float32)                # [64, S]
    sin_t = np.sin(ang).T.astype(np.float32)
    cc = np.ascontiguousarray(np.vstack([cos_t, cos_t]))    # [128, S]
    ss = np.ascontiguousarray(np.vstack([sin_t, sin_t]))

    sperm = np.zeros((128, 128), dtype=np.float32)
    for m in range(64):
        sperm[m + 64, m] = -1.0       # out[m] = -in[m+64]
        sperm[m, m + 64] = 1.0        # out[m+64] = +in[m]
    tri = np.tril(np.ones((128, 128), dtype=np.float32)).T  # tri[j,i]=1 if j<=i
    tri = np.ascontiguousarray(tri).astype(ml_dtypes.bfloat16)

    deint = np.concatenate([np.arange(0, DK, 2), np.arange(1, DK, 2)])
    in_maps = []
    for c in range(NCORES):
        bi, g = divmod(c, 2)
        heads = [g * HPC + h for h in range(HPC)]
        qk_rows = np.concatenate([hg * DK + deint for hg in heads])
        v_rows = np.arange(g * DLOC, (g + 1) * DLOC)
        in_maps.append({
            "xt": np.ascontiguousarray(x[bi].T).astype(ml_dtypes.bfloat16),
            "wq": np.ascontiguousarray(wq[qk_rows, :].T).astype(ml_dtypes.bfloat16),
            "wk": np.ascontiguousarray(wk[qk_rows, :].T).astype(ml_dtypes.bfloat16),
            "wv": np.ascontiguousarray(wv[v_rows, :].T).astype(ml_dtypes.bfloat16),
            "wo": np.ascontiguousarray(wo.T[v_rows, :]).astype(np.float32),
            "cct": cc, "sst": ss,
            "sperm": sperm, "tri": tri,
        })
    return in_maps


def assemble(results):
    out = np.empty((B, S, D), dtype=np.float32)
    for bi in range(B):
        oT = results[2 * bi]["out"] + results[2 * bi + 1]["out"]
        out[bi] = oT.T
    return out


def kernel(**inputs):
    nc = build_program()
    in_maps = prepare_in_maps(inputs["x"], inputs["wq"], inputs["wk"],
                              inputs["wv"], inputs["wo"])
    res = bass_utils.run_bass_kernel_spmd(nc, in_maps,
                                          core_ids=list(range(NCORES)))
    return assemble(res.results)


# revision 9
# speedup vs baseline: 1.1707x; 1.1707x over previous
"""Multi-head self-attention (RoPE, causal) on 8 Trainium2 NeuronCores.

Sharding: core c -> (batch = c//2, head-group = c%2 of 8 heads).
Column-parallel wq/wk/wv, row-parallel wo. Each core emits a partial
out^T [f, s]; the host sums the two partials per batch and transposes.

Layouts (all chosen so no on-device transposes are needed):
  XT  [d, s]   (x transposed on host, bf16)
  Q^T/K^T [e, s] per head from matmul(lhsT=wT[d,e], rhs=XT[d,s])
  V   [s, e]   from matmul(lhsT=XT[d,s], rhs=wvT[d,e])
  S^T [j, i] = matmul(lhsT=K^T[e,j], rhs=Q^T[e,i])
  ctx^T [e, i] = matmul(lhsT=V[j,e], rhs=expS^T[j,i])
  out^T [f, s] = matmul(lhsT=woT[d,f], rhs=ctx^T[d,s])

All matmul operands are bf16 (PSUM accumulation stays fp32); softmax
statistics and RoPE arithmetic stay fp32.

RoPE: head dims de-interleaved on host (even dims -> partitions 0..63,
odd -> 64..127 of each head's Q^T/K^T) by permuting wq/wk rows. Then
rot(x) = x*cc + (SP@x)*ss where SP is a signed permutation (matmul) and
cc/ss are host-precomputed fp32 cos/sin tables. The 1/sqrt(dk) scale is
applied via the Exp activation's scale field.

Softmax: no max-subtraction (scores are O(1)-scaled; fp32 exp is safe).
Causal masking by block-skipping + one 128x128 triangular mask on
diagonal blocks. Row sums via an all-ones [128,128] matmul (output rows
all equal the row sum, giving the partition broadcast for free);
normalization multiplies ctx^T by the DVE reciprocal of that tile.
"""

import numpy as np
import ml_dtypes

import concourse.bass as bass
import concourse.tile as tile
import concourse.mybir as mybir
from concourse import bacc, bass_utils

F32 = mybir.dt.float32
BF16 = mybir.dt.bfloat16

B = 4
S = 2048
D = 2048
NH = 16
DK = 128
NCORES = 8
HPC = 8            # heads per core
DLOC = HPC * DK    # 1024, local model dims per core
ST = S // 128      # 16 sequence 128-tiles
DT = D // 128      # 16 model-dim 128-tiles
IB = S // 512      # 4 i-blocks of 512
ROPE_THETA = 10000.0
SCALE = float(1.0 / np.sqrt(DK))

_cache = {}


def build_program():
    if "nc" in _cache:
        return _cache["nc"]

    nc = bacc.Bacc("TRN2", target_bir_lowering=False, debug=False,
                   num_devices=NCORES)

    xt = nc.dram_tensor("xt", [D, S], BF16, kind="ExternalInput").ap()
    wq = nc.dram_tensor("wq", [D, DLOC], BF16, kind="ExternalInput").ap()
    wk = nc.dram_tensor("wk", [D, DLOC], BF16, kind="ExternalInput").ap()
    wv = nc.dram_tensor("wv", [D, DLOC], BF16, kind="ExternalInput").ap()
    wo = nc.dram_tensor("wo", [DLOC, D], BF16, kind="ExternalInput").ap()
    cct = nc.dram_tensor("cct", [128, S], F32, kind="ExternalInput").ap()
    sst = nc.dram_tensor("sst", [128, S], F32, kind="ExternalInput").ap()
    sperm = nc.dram_tensor("sperm", [128, 128], BF16, kind="ExternalInput").ap()
    tri = nc.dram_tensor("tri", [128, 128], BF16, kind="ExternalInput").ap()
    out = nc.dram_tensor("out", [D, S], F32, kind="ExternalOutput").ap()

    with tile.TileContext(nc) as tc:
        with tc.tile_pool(name="dram", bufs=1, space="DRAM") as dram_pool:
            ctx_dram = dram_pool.tile([128, HPC, S], BF16)
            _attention_phase(nc, tc, xt, wq, wk, wv, cct, sst,
                             sperm, tri, ctx_dram)
            _output_phase(nc, tc, wo, ctx_dram, out)

    nc.compile()
    _cache["nc"] = nc
    return nc


def _attention_phase(nc, tc, xt, wq, wk, wv, cct, sst, sperm, tri, ctx_dram):
    with (
        tc.tile_pool(name="xt", bufs=1) as xt_pool,
        tc.tile_pool(name="vsb", bufs=1) as v_pool,
        tc.tile_pool(name="tabs", bufs=1) as tab_pool,
        tc.tile_pool(name="wqk", bufs=2) as wqk_pool,
        tc.tile_pool(name="qkraw", bufs=2) as raw_pool,
        tc.tile_pool(name="rqk", bufs=2) as rqk_pool,
        tc.tile_pool(name="qk_ps", bufs=1, space="PSUM") as qk_ps_pool,
        tc.tile_pool(name="sw_ps", bufs=2, space="PSUM") as sw_ps_pool,
    ):
        # ---- resident loads ----
        xt_sb = xt_pool.tile([128, DT, S], BF16)
        for dt in range(DT):
            nc.sync.dma_start(xt_sb[:, dt, :], xt[dt * 128:(dt + 1) * 128, :])

        cc_sb = tab_pool.tile([128, S], F32, tag="cct")
        ss_sb = tab_pool.tile([128, S], F32, tag="sst")
        nc.sync.dma_start(cc_sb[:], cct)
        nc.sync.dma_start(ss_sb[:], sst)
        sp_sb = tab_pool.tile([128, 128], BF16, tag="sperm")
        nc.sync.dma_start(sp_sb[:], sperm)
        tri_sb = tab_pool.tile([128, 128], BF16, tag="tri")
        nc.sync.dma_start(tri_sb[:], tri)
        ones_sb = tab_pool.tile([128, 128], BF16, tag="ones")
        nc.gpsimd.memset(ones_sb[:], 1.0)

        def load_wqk(h):
            wq_sb = wqk_pool.tile([128, DT, DK], BF16, tag="wq")
            wk_sb = wqk_pool.tile([128, DT, DK], BF16, tag="wk")
            for dt in range(DT):
                nc.sync.dma_start(wq_sb[:, dt, :],
                                  wq[dt * 128:(dt + 1) * 128,
                                     h * DK:(h + 1) * DK])
                nc.sync.dma_start(wk_sb[:, dt, :],
                                  wk[dt * 128:(dt + 1) * 128,
                                     h * DK:(h + 1) * DK])
            return wq_sb, wk_sb

        def proj_rope(wq_sb, wk_sb):
            rq = rqk_pool.tile([128, S], BF16, tag="rq")
            rk = rqk_pool.tile([128, S], BF16, tag="rk")
            for (w_sb, r_t) in ((wk_sb, rk), (wq_sb, rq)):
                for ch in range(4):
                    o = ch * 512
                    ps = qk_ps_pool.tile([128, 512], F32, tag="qk_ps")
                    for dt in range(DT):
                        nc.tensor.matmul(
                            ps[:],
                            w_sb[:, dt, :],
                            xt_sb[:, dt, o:o + 512],
                            start=(dt == 0), stop=(dt == DT - 1),
                        )
                    raw = raw_pool.tile([128, 512], BF16, tag="qkraw")
                    nc.scalar.copy(raw[:], ps[:])
                    swp = sw_ps_pool.tile([128, 512], F32, tag="sw_ps")
                    nc.tensor.matmul(swp[:], sp_sb[:], raw[:],
                                     start=True, stop=True)
                    t2 = raw_pool.tile([128, 512], F32, tag="t2")
                    nc.vector.tensor_mul(t2[:], swp[:], ss_sb[:, o:o + 512])
                    t3 = raw_pool.tile([128, 512], F32, tag="t3")
                    nc.vector.tensor_mul(t3[:], raw[:], cc_sb[:, o:o + 512])
                    nc.vector.tensor_add(r_t[:, o:o + 512], t2[:], t3[:])
            return rq, rk

        # head 0 QK projection first so the PE has work while wv loads
        wqk0 = load_wqk(0)
        rqk0 = proj_rope(*wqk0)

        # ---- V = x @ wv.T for all local heads: V[s, e] bf16 ----
        v_sb = v_pool.tile([128, ST, DLOC], BF16)
        with tc.tile_pool(name="v_ps", bufs=2, space="PSUM") as v_ps_pool:
            wv_sb = tab_pool.tile([128, DT, DLOC], BF16, tag="wv")
            for dt in range(DT):
                nc.sync.dma_start(wv_sb[:, dt, :],
                                  wv[dt * 128:(dt + 1) * 128, :])
            for st in range(ST):
                for g in range(2):
                    v_ps = v_ps_pool.tile([128, 512], F32, tag="v_ps")
                    for dt in range(DT):
                        nc.tensor.matmul(
                            v_ps[:],
                            xt_sb[:, dt, st * 128:(st + 1) * 128],
                            wv_sb[:, dt, g * 512:(g + 1) * 512],
                            start=(dt == 0), stop=(dt == DT - 1),
                        )
                    nc.scalar.copy(v_sb[:, st, g * 512:(g + 1) * 512], v_ps[:])

        # ---- per-head attention (+ next head's projection interleaved) ----
        with (
            tc.tile_pool(name="exps", bufs=4) as exp_pool,
            tc.tile_pool(name="small", bufs=2) as small_pool,
            tc.tile_pool(name="ctxsb", bufs=3) as ctx_sb_pool,
            tc.tile_pool(name="s_ps", bufs=2, space="PSUM") as s_ps_pool,
            tc.tile_pool(name="ctx_ps", bufs=2, space="PSUM") as ctx_ps_pool,
            tc.tile_pool(name="rs_ps", bufs=1, space="PSUM") as rs_ps_pool,
        ):
            for h in range(HPC):
                if h == 0:
                    rq, rk = rqk0
                else:
                    rq, rk = proj_rope(*load_wqk(h))

                for ib in range(IB):
                    i0 = ib * 512
                    ctx_ps = ctx_ps_pool.tile([128, 512], F32, tag="ctx_ps")
                    rs_ps = rs_ps_pool.tile([128, 512], F32, tag="rs_ps")
                    njt = 4 * ib + 4
                    for jt in range(njt):
                        r = jt - 4 * ib  # >=0 on diagonal blocks
                        lo = 128 * r if r >= 0 else 0
                        s_ps = s_ps_pool.tile([128, 512], F32, tag="s_ps")
                        nc.tensor.matmul(
                            s_ps[:],
                            rk[:, jt * 128:(jt + 1) * 128],
                            rq[:, i0:i0 + 512],
                            start=True, stop=True,
                        )
                        es = exp_pool.tile([128, 512], BF16, tag="exps")
                        nc.scalar.activation(es[:, lo:512], s_ps[:, lo:512],
                                             mybir.ActivationFunctionType.Exp,
                                             scale=SCALE)
                        if r >= 0:
                            nc.vector.tensor_mul(es[:, lo:lo + 128],
                                                 es[:, lo:lo + 128], tri_sb[:])
                        first = (jt == 0)
                        last = (jt == njt - 1)
                        nc.tensor.matmul(
                            ctx_ps[:, lo:512],
                            v_sb[:, jt, h * DK:(h + 1) * DK],
                            es[:, lo:512],
                            start=first, stop=last, skip_group_check=True,
                        )
                        nc.tensor.matmul(
                            rs_ps[:, lo:512],
                            ones_sb[:],
                            es[:, lo:512],
                            start=first, stop=last, skip_group_check=True,
                        )
                    recip = small_pool.tile([128, 512], F32, tag="recip")
                    nc.vector.reciprocal(recip[:], rs_ps[:])
                    ctx_sb = ctx_sb_pool.tile([128, 512], BF16, tag="ctx_sb")
                    nc.vector.tensor_mul(ctx_sb[:], ctx_ps[:], recip[:])
                    nc.sync.dma_start(ctx_dram[:, h, i0:i0 + 512], ctx_sb[:])


def _output_phase(nc, tc, wo, ctx_dram, out):
    NDT = DLOC // 128  # 8
    with (
        tc.tile_pool(name="wos", bufs=1) as wo_pool,
        tc.tile_pool(name="ctxin", bufs=2) as cin_pool,
        tc.tile_pool(name="outsb", bufs=3) as out_pool,
        tc.tile_pool(name="wo_ps", bufs=4, space="PSUM") as wo_ps_pool,
    ):
        wo_sb = wo_pool.tile([128, NDT, D], BF16)
        for dt in range(NDT):
            nc.sync.dma_start(wo_sb[:, dt, :], wo[dt * 128:(dt + 1) * 128, :])
        for sb4 in range(IB):
            s0 = sb4 * 512
            cin = cin_pool.tile([128, NDT, 512], BF16, tag="cin")
            nc.sync.dma_start(cin[:], ctx_dram[:, :, s0:s0 + 512])
            for ft in range(DT):
                ps = wo_ps_pool.tile([128, 512], F32, tag="wo_ps")
                for dt in range(NDT):
                    nc.tensor.matmul(
                        ps[:],
                        wo_sb[:, dt, ft * 128:(ft + 1) * 128],
                        cin[:, dt, :],
                        start=(dt == 0), stop=(dt == NDT - 1),
                    )
                osb = out_pool.tile([128, 512], F32, tag="osb")
                nc.scalar.copy(osb[:], ps[:])
                nc.sync.dma_start(out[ft * 128:(ft + 1) * 128, s0:s0 + 512],
                                  osb[:])


def prepare_in_maps(x, wq, wk, wv, wo):
    """Build the 8 per-core input maps (host-side sharding + tables)."""
    x = np.asarray(x, dtype=np.float32)
    wq = np.asarray(wq, dtype=np.float32)
    wk = np.asarray(wk, dtype=np.float32)
    wv = np.asarray(wv, dtype=np.float32)
    wo = np.asarray(wo, dtype=np.float32)

    # RoPE tables (fp32, matching the reference's fp32 cos/sin)
    f = np.arange(0, DK, 2, dtype=np.float32) / DK          # 2f/d
    inv_freq = (ROPE_THETA ** (-f)).astype(np.float32)      # [64]
    ang = np.arange(S, dtype=np.float32)[:, None] * inv_freq[None, :]
    cos_t = np.cos(ang).T.astype(np.float32)                # [64, S]
    sin_t = np.sin(ang).T.astype(np.float32)
    cc = np.ascontiguousarray(np.vstack([cos_t, cos_t]))    # [128, S]
    ss = np.ascontiguousarray(np.vstack([sin_t, sin_t]))

    sperm = np.zeros((128, 128), dtype=np.float32)
    for m in range(64):
        sperm[m + 64, m] = -1.0       # out[m] = -in[m+64]
        sperm[m, m + 64] = 1.0        # out[m+64] = +in[m]
    sperm = sperm.astype(ml_dtypes.bfloat16)
    tri = np.tril(np.ones((128, 128), dtype=np.float32)).T  # tri[j,i]=1 if j<=i
    tri = np.ascontiguousarray(tri).astype(ml_dtypes.bfloat16)

    deint = np.concatenate([np.arange(0, DK, 2), np.arange(1, DK, 2)])
    in_maps = []
    for c in range(NCORES):
        bi, g = divmod(c, 2)
        heads = [g * HPC + h for h in range(HPC)]
        qk_rows = np.concatenate([hg * DK + deint for hg in heads])
        v_rows = np.arange(g * DLOC, (g + 1) * DLOC)
        in_maps.append({
            "xt": np.ascontiguousarray(x[bi].T).astype(ml_dtypes.bfloat16),
            "wq": np.ascontiguousarray(wq[qk_rows, :].T).astype(ml_dtypes.bfloat16),
            "wk": np.ascontiguousarray(wk[qk_rows, :].T).astype(ml_dtypes.bfloat16),
            "wv": np.ascontiguousarray(wv[v_rows, :].T).astype(ml_dtypes.bfloat16),
            "wo": np.ascontiguousarray(wo.T[v_rows, :]).astype(ml_dtypes.bfloat16),
            "cct": cc, "sst": ss,
            "sperm": sperm, "tri": tri,
        })
    return in_maps


def assemble(results):
    out = np.empty((B, S, D), dtype=np.float32)
    for bi in range(B):
        oT = results[2 * bi]["out"] + results[2 * bi + 1]["out"]
        out[bi] = oT.T
    return out


def kernel(**inputs):
    nc = build_program()
    in_maps = prepare_in_maps(inputs["x"], inputs["wq"], inputs["wk"],
                              inputs["wv"], inputs["wo"])
    res = bass_utils.run_bass_kernel_spmd(nc, in_maps,
                                          core_ids=list(range(NCORES)))
    return assemble(res.results)


# revision 10
# speedup vs baseline: 1.2795x; 1.0929x over previous
"""Multi-head self-attention (RoPE, causal) on 8 Trainium2 NeuronCores.

Sharding: core c -> (batch = c//2, head-group = c%2 of 8 heads).
Column-parallel wq/wk/wv, row-parallel wo. Each core emits a partial
out^T [f, s]; the host sums the two partials per batch and transposes.

Layouts (all chosen so no on-device transposes are needed):
  XT  [d, s]   (x transposed on host, bf16)
  Q^T/K^T [e, s] per head from matmul(lhsT=wT[d,e], rhs=XT[d,s])
  V   [s, e]   from matmul(lhsT=XT[d,s], rhs=wvT[d,e])
  S^T [j, i] = matmul(lhsT=K^T[e,j], rhs=Q^T[e,i])
  ctx^T [e, i] = matmul(lhsT=V[j,e], rhs=expS^T[j,i])
  out^T [f, s] = matmul(lhsT=woT[d,f], rhs=ctx^T[d,s])

All matmul operands are bf16 (PSUM accumulation stays fp32); softmax
statistics and RoPE arithmetic stay fp32.

RoPE: head dims de-interleaved on host (even dims -> partitions 0..63,
odd -> 64..127 of each head's Q^T/K^T) by permuting wq/wk rows. Then
rot(x) = x*cc + (SP@x)*ss where SP is a signed permutation (matmul) and
cc/ss are host-precomputed fp32 cos/sin tables. The 1/sqrt(dk) scale is
applied via the Exp activation's scale field.

Softmax: no max-subtraction (scores are O(1)-scaled; fp32 exp is safe).
Causal masking by block-skipping + one 128x128 triangular mask on
diagonal blocks. Row sums via an all-ones [128,128] matmul (output rows
all equal the row sum, giving the partition broadcast for free);
normalization multiplies ctx^T by the DVE reciprocal of that tile.
"""

import numpy as np
import ml_dtypes

import concourse.bass as bass
import concourse.tile as tile
import concourse.mybir as mybir
from concourse import bacc, bass_utils

F32 = mybir.dt.float32
BF16 = mybir.dt.bfloat16

B = 4
S = 2048
D = 2048
NH = 16
DK = 128
NCORES = 8
HPC = 8            # heads per core
DLOC = HPC * DK    # 1024, local model dims per core
ST = S // 128      # 16 sequence 128-tiles
DT = D // 128      # 16 model-dim 128-tiles
IB = S // 512      # 4 i-blocks of 512
ROPE_THETA = 10000.0
SCALE = float(1.0 / np.sqrt(DK))

_cache = {}


def build_program():
    if "nc" in _cache:
        return _cache["nc"]

    nc = bacc.Bacc("TRN2", target_bir_lowering=False, debug=False,
                   num_devices=NCORES)

    xt = nc.dram_tensor("xt", [D, S], BF16, kind="ExternalInput").ap()
    wq = nc.dram_tensor("wq", [D, DLOC], BF16, kind="ExternalInput").ap()
    wk = nc.dram_tensor("wk", [D, DLOC], BF16, kind="ExternalInput").ap()
    wv = nc.dram_tensor("wv", [D, DLOC], BF16, kind="ExternalInput").ap()
    wo = nc.dram_tensor("wo", [DLOC, D], BF16, kind="ExternalInput").ap()
    cct = nc.dram_tensor("cct", [128, S], F32, kind="ExternalInput").ap()
    sst = nc.dram_tensor("sst", [128, S], F32, kind="ExternalInput").ap()
    sperm = nc.dram_tensor("sperm", [128, 128], BF16, kind="ExternalInput").ap()
    tri = nc.dram_tensor("tri", [128, 128], BF16, kind="ExternalInput").ap()
    out = nc.dram_tensor("out", [D, S], F32, kind="ExternalOutput").ap()

    with tile.TileContext(nc) as tc:
        with tc.tile_pool(name="dram", bufs=1, space="DRAM") as dram_pool:
            ctx_dram = dram_pool.tile([128, HPC, S], BF16)
            _attention_phase(nc, tc, xt, wq, wk, wv, cct, sst,
                             sperm, tri, ctx_dram)
            _output_phase(nc, tc, wo, ctx_dram, out)

    nc.compile()
    _cache["nc"] = nc
    return nc


def _attention_phase(nc, tc, xt, wq, wk, wv, cct, sst, sperm, tri, ctx_dram):
    with (
        tc.tile_pool(name="xt", bufs=1) as xt_pool,
        tc.tile_pool(name="vsb", bufs=1) as v_pool,
        tc.tile_pool(name="tabs", bufs=1) as tab_pool,
        tc.tile_pool(name="wqk", bufs=2) as wqk_pool,
        tc.tile_pool(name="qkraw", bufs=2) as raw_pool,
        tc.tile_pool(name="rqk", bufs=2) as rqk_pool,
        tc.tile_pool(name="qk_ps", bufs=1, space="PSUM") as qk_ps_pool,
        tc.tile_pool(name="sw_ps", bufs=1, space="PSUM") as sw_ps_pool,
    ):
        # ---- resident loads ----
        xt_sb = xt_pool.tile([128, DT, S], BF16)
        for dt in range(DT):
            nc.sync.dma_start(xt_sb[:, dt, :], xt[dt * 128:(dt + 1) * 128, :])

        cc_sb = tab_pool.tile([128, S], F32, tag="cct")
        ss_sb = tab_pool.tile([128, S], F32, tag="sst")
        nc.sync.dma_start(cc_sb[:], cct)
        nc.sync.dma_start(ss_sb[:], sst)
        sp_sb = tab_pool.tile([128, 128], BF16, tag="sperm")
        nc.sync.dma_start(sp_sb[:], sperm)
        tri_sb = tab_pool.tile([128, 128], BF16, tag="tri")
        nc.sync.dma_start(tri_sb[:], tri)
        ones_sb = tab_pool.tile([128, 128], BF16, tag="ones")
        nc.gpsimd.memset(ones_sb[:], 1.0)

        def load_wqk(h):
            wq_sb = wqk_pool.tile([128, DT, DK], BF16, tag="wq")
            wk_sb = wqk_pool.tile([128, DT, DK], BF16, tag="wk")
            for dt in range(DT):
                nc.sync.dma_start(wq_sb[:, dt, :],
                                  wq[dt * 128:(dt + 1) * 128,
                                     h * DK:(h + 1) * DK])
                nc.sync.dma_start(wk_sb[:, dt, :],
                                  wk[dt * 128:(dt + 1) * 128,
                                     h * DK:(h + 1) * DK])
            return wq_sb, wk_sb

        def proj_rope(wq_sb, wk_sb):
            rq = rqk_pool.tile([128, S], BF16, tag="rq")
            rk = rqk_pool.tile([128, S], BF16, tag="rk")
            for (w_sb, r_t) in ((wk_sb, rk), (wq_sb, rq)):
                for ch in range(4):
                    o = ch * 512
                    ps = qk_ps_pool.tile([128, 512], F32, tag="qk_ps")
                    for dt in range(DT):
                        nc.tensor.matmul(
                            ps[:],
                            w_sb[:, dt, :],
                            xt_sb[:, dt, o:o + 512],
                            start=(dt == 0), stop=(dt == DT - 1),
                        )
                    raw = raw_pool.tile([128, 512], BF16, tag="qkraw")
                    nc.scalar.copy(raw[:], ps[:])
                    swp = sw_ps_pool.tile([128, 512], F32, tag="sw_ps")
                    nc.tensor.matmul(swp[:], sp_sb[:], raw[:],
                                     start=True, stop=True)
                    t2 = raw_pool.tile([128, 512], F32, tag="t2")
                    nc.vector.tensor_mul(t2[:], swp[:], ss_sb[:, o:o + 512])
                    t3 = raw_pool.tile([128, 512], F32, tag="t3")
                    nc.vector.tensor_mul(t3[:], raw[:], cc_sb[:, o:o + 512])
                    nc.vector.tensor_add(r_t[:, o:o + 512], t2[:], t3[:])
            return rq, rk

        # head 0 QK projection first so the PE has work while wv loads
        wqk0 = load_wqk(0)
        rqk0 = proj_rope(*wqk0)

        # ---- V = x @ wv.T for all local heads: V[s, e] bf16 ----
        v_sb = v_pool.tile([128, ST, DLOC], BF16)
        with tc.tile_pool(name="v_ps", bufs=2, space="PSUM") as v_ps_pool:
            wv_sb = tab_pool.tile([128, DT, DLOC], BF16, tag="wv")
            for dt in range(DT):
                nc.sync.dma_start(wv_sb[:, dt, :],
                                  wv[dt * 128:(dt + 1) * 128, :])
            for st in range(ST):
                for g in range(2):
                    v_ps = v_ps_pool.tile([128, 512], F32, tag="v_ps")
                    for dt in range(DT):
                        nc.tensor.matmul(
                            v_ps[:],
                            xt_sb[:, dt, st * 128:(st + 1) * 128],
                            wv_sb[:, dt, g * 512:(g + 1) * 512],
                            start=(dt == 0), stop=(dt == DT - 1),
                        )
                    nc.scalar.copy(v_sb[:, st, g * 512:(g + 1) * 512], v_ps[:])

        # ---- per-head attention (+ next head's projection interleaved) ----
        with (
            tc.tile_pool(name="exps", bufs=4) as exp_pool,
            tc.tile_pool(name="small", bufs=2) as small_pool,
            tc.tile_pool(name="ctxsb", bufs=3) as ctx_sb_pool,
            tc.tile_pool(name="s_ps", bufs=2, space="PSUM") as s_ps_pool,
            tc.tile_pool(name="ctx_ps", bufs=2, space="PSUM") as ctx_ps_pool,
            tc.tile_pool(name="rs_ps", bufs=2, space="PSUM") as rs_ps_pool,
        ):
            for h in range(HPC):
                if h == 0:
                    rq, rk = rqk0
                else:
                    rq, rk = proj_rope(*load_wqk(h))

                for ib in range(IB):
                    i0 = ib * 512
                    ctx_ps = ctx_ps_pool.tile([128, 512], F32, tag="ctx_ps")
                    rs_ps = rs_ps_pool.tile([128, 512], F32, tag="rs_ps")
                    njt = 4 * ib + 4
                    for jt in range(njt):
                        r = jt - 4 * ib  # >=0 on diagonal blocks
                        lo = 128 * r if r >= 0 else 0
                        s_ps = s_ps_pool.tile([128, 512], F32, tag="s_ps")
                        nc.tensor.matmul(
                            s_ps[:],
                            rk[:, jt * 128:(jt + 1) * 128],
                            rq[:, i0:i0 + 512],
                            start=True, stop=True,
                        )
                        es = exp_pool.tile([128, 512], BF16, tag="exps")
                        nc.scalar.activation(es[:, lo:512], s_ps[:, lo:512],
                                             mybir.ActivationFunctionType.Exp,
                                             scale=SCALE)
                        if r >= 0:
                            nc.vector.tensor_mul(es[:, lo:lo + 128],
                                                 es[:, lo:lo + 128], tri_sb[:])
                        first = (jt == 0)
                        last = (jt == njt - 1)
                        nc.tensor.matmul(
                            ctx_ps[:, lo:512],
                            v_sb[:, jt, h * DK:(h + 1) * DK],
                            es[:, lo:512],
                            start=first, stop=last, skip_group_check=True,
                        )
                        nc.tensor.matmul(
                            rs_ps[:, lo:512],
                            ones_sb[:],
                            es[:, lo:512],
                            start=first, stop=last, skip_group_check=True,
                        )
                    recip = small_pool.tile([128, 512], F32, tag="recip")
                    nc.vector.reciprocal_approx_fast(recip[:], rs_ps[:])
                    ctx_sb = ctx_sb_pool.tile([128, 512], BF16, tag="ctx_sb")
                    nc.vector.tensor_mul(ctx_sb[:], ctx_ps[:], recip[:])
                    nc.sync.dma_start(ctx_dram[:, h, i0:i0 + 512], ctx_sb[:])


def _output_phase(nc, tc, wo, ctx_dram, out):
    NDT = DLOC // 128  # 8
    with (
        tc.tile_pool(name="wos", bufs=1) as wo_pool,
        tc.tile_pool(name="ctxin", bufs=2) as cin_pool,
        tc.tile_pool(name="outsb", bufs=3) as out_pool,
        tc.tile_pool(name="wo_ps", bufs=4, space="PSUM") as wo_ps_pool,
    ):
        wo_sb = wo_pool.tile([128, NDT, D], BF16)
        for dt in range(NDT):
            nc.sync.dma_start(wo_sb[:, dt, :], wo[dt * 128:(dt + 1) * 128, :])
        for sb4 in range(IB):
            s0 = sb4 * 512
            cin = cin_pool.tile([128, NDT, 512], BF16, tag="cin")
            nc.sync.dma_start(cin[:], ctx_dram[:, :, s0:s0 + 512])
            for ft in range(DT):
                ps = wo_ps_pool.tile([128, 512], F32, tag="wo_ps")
                for dt in range(NDT):
                    nc.tensor.matmul(
                        ps[:],
                        wo_sb[:, dt, ft * 128:(ft + 1) * 128],
                        cin[:, dt, :],
                        start=(dt == 0), stop=(dt == NDT - 1),
                    )
                osb = out_pool.tile([128, 512], F32, tag="osb")
                nc.scalar.copy(osb[:], ps[:])
                nc.sync.dma_start(out[ft * 128:(ft + 1) * 128, s0:s0 + 512],
                                  osb[:])


def prepare_in_maps(x, wq, wk, wv, wo):
    """Build the 8 per-core input maps (host-side sharding + tables)."""
    x = np.asarray(x, dtype=np.float32)
    wq = np.asarray(wq, dtype=np.float32)
    wk = np.asarray(wk, dtype=np.float32)
    wv = np.asarray(wv, dtype=np.float32)
    wo = np.asarray(wo, dtype=np.float32)

    # RoPE tables (fp32, matching the reference's fp32 cos/sin)
    f = np.arange(0, DK, 2, dtype=np.float32) / DK          # 2f/d
    inv_freq = (ROPE_THETA ** (-f)).astype(np.float32)      # [64]
    ang = np.arange(S, dtype=np.float32)[:, None] * inv_freq[None, :]
    cos_t = np.cos(ang).T.astype(np.float32)                # [64, S]
    sin_t = np.sin(ang).T.astype(np.float32)
    cc = np.ascontiguousarray(np.vstack([cos_t, cos_t]))    # [128, S]
    ss = np.ascontiguousarray(np.vstack([sin_t, sin_t]))

    sperm = np.zeros((128, 128), dtype=np.float32)
    for m in range(64):
        sperm[m + 64, m] = -1.0       # out[m] = -in[m+64]
        sperm[m, m + 64] = 1.0        # out[m+64] = +in[m]
    sperm = sperm.astype(ml_dtypes.bfloat16)
    tri = np.tril(np.ones((128, 128), dtype=np.float32)).T  # tri[j,i]=1 if j<=i
    tri = np.ascontiguousarray(tri).astype(ml_dtypes.bfloat16)

    deint = np.concatenate([np.arange(0, DK, 2), np.arange(1, DK, 2)])
    in_maps = []
    for c in range(NCORES):
        bi, g = divmod(c, 2)
        heads = [g * HPC + h for h in range(HPC)]
        qk_rows = np.concatenate([hg * DK + deint for hg in heads])
        v_rows = np.arange(g * DLOC, (g + 1) * DLOC)
        in_maps.append({
            "xt": np.ascontiguousarray(x[bi].T).astype(ml_dtypes.bfloat16),
            "wq": np.ascontiguousarray(wq[qk_rows, :].T).astype(ml_dtypes.bfloat16),
            "wk": np.ascontiguousarray(wk[qk_rows, :].T).astype(ml_dtypes.bfloat16),
            "wv": np.ascontiguousarray(wv[v_rows, :].T).astype(ml_dtypes.bfloat16),
            "wo": np.ascontiguousarray(wo.T[v_rows, :]).astype(ml_dtypes.bfloat16),
            "cct": cc, "sst": ss,
            "sperm": sperm, "tri": tri,
        })
    return in_maps


def assemble(results):
    out = np.empty((B, S, D), dtype=np.float32)
    for bi in range(B):
        oT = results[2 * bi]["out"] + results[2 * bi + 1]["out"]
        out[bi] = oT.T
    return out


def kernel(**inputs):
    nc = build_program()
    in_maps = prepare_in_maps(inputs["x"], inputs["wq"], inputs["wk"],
                              inputs["wv"], inputs["wo"])
    res = bass_utils.run_bass_kernel_spmd(nc, in_maps,
                                          core_ids=list(range(NCORES)))
    return assemble(res.results)


# revision 13
# speedup vs baseline: 1.2918x; 1.0096x over previous
"""Multi-head self-attention (RoPE, causal) on 8 Trainium2 NeuronCores.

Sharding: core c -> (batch = c//2, head-group = c%2 of 8 heads).
Column-parallel wq/wk/wv, row-parallel wo. Each core emits a partial
out^T [f, s]; the host sums the two partials per batch and transposes.

Layouts (all chosen so no on-device transposes are needed):
  XT  [d, s]   (x transposed on host, bf16)
  Q^T/K^T [e, s] per head from matmul(lhsT=wT[d,e], rhs=XT[d,s])
  V   [s, e]   from matmul(lhsT=XT[d,s], rhs=wvT[d,e])
  S^T [j, i] = matmul(lhsT=K^T[e,j], rhs=Q^T[e,i])
  ctx^T [e, i] = matmul(lhsT=V[j,e], rhs=expS^T[j,i])
  out^T [f, s] = matmul(lhsT=woT[d,f], rhs=ctx^T[d,s])

All matmul operands are bf16 (PSUM accumulation stays fp32); softmax
statistics and RoPE arithmetic stay fp32.

RoPE: head dims de-interleaved on host (even dims -> partitions 0..63,
odd -> 64..127 of each head's Q^T/K^T) by permuting wq/wk rows. Then
rot(x) = x*cc + (SP@x)*ss where SP is a signed permutation (matmul) and
cc/ss are host-precomputed fp32 cos/sin tables. The 1/sqrt(dk) scale is
applied via the Exp activation's scale field.

Softmax: no max-subtraction (scores are O(1)-scaled; fp32 exp is safe).
Causal masking by block-skipping + one 128x128 triangular mask on
diagonal blocks. Row sums via an all-ones [128,128] matmul (output rows
all equal the row sum, giving the partition broadcast for free);
normalization multiplies ctx^T by the DVE reciprocal of that tile.
"""

import numpy as np
import ml_dtypes

import concourse.bass as bass
import concourse.tile as tile
import concourse.mybir as mybir
from concourse import bacc, bass_utils

F32 = mybir.dt.float32
BF16 = mybir.dt.bfloat16

B = 4
S = 2048
D = 2048
NH = 16
DK = 128
NCORES = 8
HPC = 8            # heads per core
DLOC = HPC * DK    # 1024, local model dims per core
ST = S // 128      # 16 sequence 128-tiles
DT = D // 128      # 16 model-dim 128-tiles
IB = S // 512      # 4 i-blocks of 512
ROPE_THETA = 10000.0
SCALE = float(1.0 / np.sqrt(DK))

_cache = {}


def build_program():
    if "nc" in _cache:
        return _cache["nc"]

    nc = bacc.Bacc("TRN2", target_bir_lowering=False, debug=False,
                   num_devices=NCORES)

    xt = nc.dram_tensor("xt", [D, S], BF16, kind="ExternalInput").ap()
    wq = nc.dram_tensor("wq", [D, DLOC], BF16, kind="ExternalInput").ap()
    wk = nc.dram_tensor("wk", [D, DLOC], BF16, kind="ExternalInput").ap()
    wv = nc.dram_tensor("wv", [D, DLOC], BF16, kind="ExternalInput").ap()
    wo = nc.dram_tensor("wo", [DLOC, D], BF16, kind="ExternalInput").ap()
    cct = nc.dram_tensor("cct", [128, S], F32, kind="ExternalInput").ap()
    sst = nc.dram_tensor("sst", [128, S], F32, kind="ExternalInput").ap()
    sperm = nc.dram_tensor("sperm", [128, 128], BF16, kind="ExternalInput").ap()
    tri = nc.dram_tensor("tri", [128, 128], BF16, kind="ExternalInput").ap()
    out = nc.dram_tensor("out", [D, S], F32, kind="ExternalOutput").ap()

    with tile.TileContext(nc) as tc:
        with tc.tile_pool(name="dram", bufs=1, space="DRAM") as dram_pool:
            ctx_dram = dram_pool.tile([128, HPC, S], BF16)
            _attention_phase(nc, tc, xt, wq, wk, wv, cct, sst,
                             sperm, tri, ctx_dram)
            _output_phase(nc, tc, wo, ctx_dram, out)

    nc.compile()
    _cache["nc"] = nc
    return nc


def _attention_phase(nc, tc, xt, wq, wk, wv, cct, sst, sperm, tri, ctx_dram):
    with (
        tc.tile_pool(name="xt", bufs=1) as xt_pool,
        tc.tile_pool(name="vsb", bufs=1) as v_pool,
        tc.tile_pool(name="tabs", bufs=1) as tab_pool,
        tc.tile_pool(name="wqk", bufs=2) as wqk_pool,
        tc.tile_pool(name="qkraw", bufs=2) as raw_pool,
        tc.tile_pool(name="rqk", bufs=2) as rqk_pool,
        tc.tile_pool(name="qk_ps", bufs=1, space="PSUM") as qk_ps_pool,
        tc.tile_pool(name="sw_ps", bufs=1, space="PSUM") as sw_ps_pool,
    ):
        # ---- resident loads ----
        def load_wqk(h):
            wq_sb = wqk_pool.tile([128, DT, DK], BF16, tag="wq")
            wk_sb = wqk_pool.tile([128, DT, DK], BF16, tag="wk")
            for dt in range(DT):
                nc.sync.dma_start(wq_sb[:, dt, :],
                                  wq[dt * 128:(dt + 1) * 128,
                                     h * DK:(h + 1) * DK])
                nc.sync.dma_start(wk_sb[:, dt, :],
                                  wk[dt * 128:(dt + 1) * 128,
                                     h * DK:(h + 1) * DK])
            return wq_sb, wk_sb

        wqk0 = load_wqk(0)
        # xt chunked column-major so head-0's first projection chunk can
        # start after ~2MB instead of the full 8.4MB load
        xt_sb = xt_pool.tile([128, DT, S], BF16)
        for ch in range(4):
            for dt in range(DT):
                nc.sync.dma_start(
                    xt_sb[:, dt, ch * 512:(ch + 1) * 512],
                    xt[dt * 128:(dt + 1) * 128, ch * 512:(ch + 1) * 512])

        cc_sb = tab_pool.tile([128, S], F32, tag="cct")
        ss_sb = tab_pool.tile([128, S], F32, tag="sst")
        nc.sync.dma_start(cc_sb[:], cct)
        nc.sync.dma_start(ss_sb[:], sst)
        sp_sb = tab_pool.tile([128, 128], BF16, tag="sperm")
        nc.sync.dma_start(sp_sb[:], sperm)
        tri_sb = tab_pool.tile([128, 128], BF16, tag="tri")
        nc.sync.dma_start(tri_sb[:], tri)
        ones_sb = tab_pool.tile([128, 128], BF16, tag="ones")
        nc.gpsimd.memset(ones_sb[:], 1.0)

        def proj_rope(wq_sb, wk_sb):
            rq = rqk_pool.tile([128, S], BF16, tag="rq")
            rk = rqk_pool.tile([128, S], BF16, tag="rk")
            for ch in range(4):
                for (w_sb, r_t) in ((wk_sb, rk), (wq_sb, rq)):
                    o = ch * 512
                    ps = qk_ps_pool.tile([128, 512], F32, tag="qk_ps")
                    for dt in range(DT):
                        nc.tensor.matmul(
                            ps[:],
                            w_sb[:, dt, :],
                            xt_sb[:, dt, o:o + 512],
                            start=(dt == 0), stop=(dt == DT - 1),
                        )
                    raw = raw_pool.tile([128, 512], BF16, tag="qkraw")
                    nc.scalar.copy(raw[:], ps[:])
                    swp = sw_ps_pool.tile([128, 512], F32, tag="sw_ps")
                    nc.tensor.matmul(swp[:], sp_sb[:], raw[:],
                                     start=True, stop=True)
                    t2 = raw_pool.tile([128, 512], F32, tag="t2")
                    nc.vector.tensor_mul(t2[:], swp[:], ss_sb[:, o:o + 512])
                    t3 = raw_pool.tile([128, 512], F32, tag="t3")
                    nc.vector.tensor_mul(t3[:], raw[:], cc_sb[:, o:o + 512])
                    nc.vector.tensor_add(r_t[:, o:o + 512], t2[:], t3[:])
            return rq, rk

        # head 0 QK projection first so the PE has work while wv loads
        rqk0 = proj_rope(*wqk0)

        # ---- V = x @ wv.T for all local heads: V[s, e] bf16 ----
        v_sb = v_pool.tile([128, ST, DLOC], BF16)
        with tc.tile_pool(name="v_ps", bufs=2, space="PSUM") as v_ps_pool:
            wv_sb = tab_pool.tile([128, DT, DLOC], BF16, tag="wv")
            for dt in range(DT):
                nc.sync.dma_start(wv_sb[:, dt, :],
                                  wv[dt * 128:(dt + 1) * 128, :])
            for st in range(ST):
                for g in range(2):
                    v_ps = v_ps_pool.tile([128, 512], F32, tag="v_ps")
                    for dt in range(DT):
                        nc.tensor.matmul(
                            v_ps[:],
                            xt_sb[:, dt, st * 128:(st + 1) * 128],
                            wv_sb[:, dt, g * 512:(g + 1) * 512],
                            start=(dt == 0), stop=(dt == DT - 1),
                        )
                    nc.scalar.copy(v_sb[:, st, g * 512:(g + 1) * 512], v_ps[:])

        # ---- per-head attention (+ next head's projection interleaved) ----
        with (
            tc.tile_pool(name="exps", bufs=4) as exp_pool,
            tc.tile_pool(name="small", bufs=2) as small_pool,
            tc.tile_pool(name="ctxsb", bufs=3) as ctx_sb_pool,
            tc.tile_pool(name="s_ps", bufs=2, space="PSUM") as s_ps_pool,
            tc.tile_pool(name="ctx_ps", bufs=2, space="PSUM") as ctx_ps_pool,
            tc.tile_pool(name="rs_ps", bufs=2, space="PSUM") as rs_ps_pool,
        ):
            for h in range(HPC):
                if h == 0:
                    rq, rk = rqk0
                else:
                    rq, rk = proj_rope(*load_wqk(h))

                for ib in range(IB):
                    i0 = ib * 512
                    ctx_ps = ctx_ps_pool.tile([128, 512], F32, tag="ctx_ps")
                    rs_ps = rs_ps_pool.tile([128, 512], F32, tag="rs_ps")
                    njt = 4 * ib + 4
                    for jt in range(njt):
                        r = jt - 4 * ib  # >=0 on diagonal blocks
                        lo = 128 * r if r >= 0 else 0
                        s_ps = s_ps_pool.tile([128, 512], F32, tag="s_ps")
                        nc.tensor.matmul(
                            s_ps[:],
                            rk[:, jt * 128:(jt + 1) * 128],
                            rq[:, i0:i0 + 512],
                            start=True, stop=True,
                        )
                        es = exp_pool.tile([128, 512], BF16, tag="exps")
                        nc.scalar.activation(es[:, lo:512], s_ps[:, lo:512],
                                             mybir.ActivationFunctionType.Exp,
                                             scale=SCALE)
                        if r >= 0:
                            nc.vector.tensor_mul(es[:, lo:lo + 128],
                                                 es[:, lo:lo + 128], tri_sb[:])
                        first = (jt == 0)
                        last = (jt == njt - 1)
                        nc.tensor.matmul(
                            ctx_ps[:, lo:512],
                            v_sb[:, jt, h * DK:(h + 1) * DK],
                            es[:, lo:512],
                            start=first, stop=last, skip_group_check=True,
                        )
                        nc.tensor.matmul(
                            rs_ps[:, lo:512],
                            ones_sb[:],
                            es[:, lo:512],
                            start=first, stop=last, skip_group_check=True,
                        )
                    recip = small_pool.tile([128, 512], F32, tag="recip")
                    nc.vector.reciprocal_approx_fast(recip[:], rs_ps[:])
                    ctx_sb = ctx_sb_pool.tile([128, 512], BF16, tag="ctx_sb")
                    nc.vector.tensor_mul(ctx_sb[:], ctx_ps[:], recip[:])
                    nc.sync.dma_start(ctx_dram[:, h, i0:i0 + 512], ctx_sb[:])


def _output_phase(nc, tc, wo, ctx_dram, out):
    NDT = DLOC // 128  # 8
    with (
        tc.tile_pool(name="wos", bufs=1) as wo_pool,
        tc.tile_pool(name="ctxin", bufs=2) as cin_pool,
        tc.tile_pool(name="outsb", bufs=3) as out_pool,
        tc.tile_pool(name="wo_ps", bufs=4, space="PSUM") as wo_ps_pool,
    ):
        wo_sb = wo_pool.tile([128, NDT, D], BF16)
        for dt in range(NDT):
            nc.sync.dma_start(wo_sb[:, dt, :], wo[dt * 128:(dt + 1) * 128, :])
        for sb4 in range(IB):
            s0 = sb4 * 512
            cin = cin_pool.tile([128, NDT, 512], BF16, tag="cin")
            nc.sync.dma_start(cin[:], ctx_dram[:, :, s0:s0 + 512])
            for ft in range(DT):
                ps = wo_ps_pool.tile([128, 512], F32, tag="wo_ps")
                for dt in range(NDT):
                    nc.tensor.matmul(
                        ps[:],
                        wo_sb[:, dt, ft * 128:(ft + 1) * 128],
                        cin[:, dt, :],
                        start=(dt == 0), stop=(dt == NDT - 1),
                    )
                osb = out_pool.tile([128, 512], F32, tag="osb")
                nc.scalar.copy(osb[:], ps[:])
                nc.sync.dma_start(out[ft * 128:(ft + 1) * 128, s0:s0 + 512],
                                  osb[:])


def prepare_in_maps(x, wq, wk, wv, wo):
    """Build the 8 per-core input maps (host-side sharding + tables)."""
    x = np.asarray(x, dtype=np.float32)
    wq = np.asarray(wq, dtype=np.float32)
    wk = np.asarray(wk, dtype=np.float32)
    wv = np.asarray(wv, dtype=np.float32)
    wo = np.asarray(wo, dtype=np.float32)

    # RoPE tables (fp32, matching the reference's fp32 cos/sin)
    f = np.arange(0, DK, 2, dtype=np.float32) / DK          # 2f/d
    inv_freq = (ROPE_THETA ** (-f)).astype(np.float32)      # [64]
    ang = np.arange(S, dtype=np.float32)[:, None] * inv_freq[None, :]
    cos_t = np.cos(ang).T.astype(np.float32)                # [64, S]
    sin_t = np.sin(ang).T.astype(np.float32)
    cc = np.ascontiguousarray(np.vstack([cos_t, cos_t]))    # [128, S]
    ss = np.ascontiguousarray(np.vstack([sin_t, sin_t]))

    sperm = np.zeros((128, 128), dtype=np.float32)
    for m in range(64):
        sperm[m + 64, m] = -1.0       # out[m] = -in[m+64]
        sperm[m, m + 64] = 1.0        # out[m+64] = +in[m]
    sperm = sperm.astype(ml_dtypes.bfloat16)
    tri = np.tril(np.ones((128, 128), dtype=np.float32)).T  # tri[j,i]=1 if j<=i
    tri = np.ascontiguousarray(tri).astype(ml_dtypes.bfloat16)

    deint = np.concatenate([np.arange(0, DK, 2), np.arange(1, DK, 2)])
    in_maps = []
    for c in range(NCORES):
        bi, g = divmod(c, 2)
        heads = [g * HPC + h for h in range(HPC)]
        qk_rows = np.concatenate([hg * DK + deint for hg in heads])
        v_rows = np.arange(g * DLOC, (g + 1) * DLOC)
        in_maps.append({
            "xt": np.ascontiguousarray(x[bi].T).astype(ml_dtypes.bfloat16),
            "wq": np.ascontiguousarray(wq[qk_rows, :].T).astype(ml_dtypes.bfloat16),
            "wk": np.ascontiguousarray(wk[qk_rows, :].T).astype(ml_dtypes.bfloat16),
            "wv": np.ascontiguousarray(wv[v_rows, :].T).astype(ml_dtypes.bfloat16),
            "wo": np.ascontiguousarray(wo.T[v_rows, :]).astype(ml_dtypes.bfloat16),
            "cct": cc, "sst": ss,
            "sperm": sperm, "tri": tri,
        })
    return in_maps


def assemble(results):
    out = np.empty((B, S, D), dtype=np.float32)
    for bi in range(B):
        oT = results[2 * bi]["out"] + results[2 * bi + 1]["out"]
        out[bi] = oT.T
    return out


def kernel(**inputs):
    nc = build_program()
    in_maps = prepare_in_maps(inputs["x"], inputs["wq"], inputs["wk"],
                              inputs["wv"], inputs["wo"])
    res = bass_utils.run_bass_kernel_spmd(nc, in_maps,
                                          core_ids=list(range(NCORES)))
    return assemble(res.results)
